# revision 7
# baseline (speedup 1.0000x reference)
"""HDC image classifier predict: features -> binary hypervectors -> hamming
similarity vs class prototypes -> (preds, similarities).

Strategy (8 NeuronCores, data-parallel over the batch):
  - Each core gets 2048 of the 16384 feature rows; random_projection and
    prototypes are replicated.
  - projection matmul runs on the PE array as two bf16 passes (hi + lo split
    of the features) accumulating in fp32 PSUM: random_projection is exactly
    representable in bf16 (entries are +-1), so the split recovers ~fp32
    precision at bf16 speed.
  - hv bits are thresholded on the Vector engine straight out of PSUM, and a
    second PE matmul against prototypes^T (with an appended ones column for
    the popcount) accumulates hamming cross terms over all of D.
  - similarities and the argmax (Vector engine max/max_index, first-occurrence
    semantics matching jnp.argmax) are computed on-device; outputs are
    gathered on the host.
"""

from contextlib import ExitStack

import numpy as np
import ml_dtypes

B, F, D, C = 16384, 512, 10000, 100
NCORES = 8
BL = B // NCORES            # 2048 rows per core
DP = 10112                  # D padded to 79*128
ND = DP // 128              # 79 d-chunks
NF = F // 128               # 4 f-chunks of the contraction dim
NBS = 4                     # batch super-chunks per core
BS = BL // NBS              # 512 rows per super-chunk
NBB = BS // 128             # 4 row-blocks per super-chunk
CP = C + 1                  # classes + popcount column
RPC = 10                    # d-chunks per rp DMA piece
NRPC = (ND + RPC - 1) // RPC  # 8 pieces

_CACHE = {}


def _build_module():
    import concourse.tile as tile
    import concourse.mybir as mybir
    from concourse import bacc
    from concourse.masks import make_identity

    dt = mybir.dt
    op = mybir.AluOpType

    nc = bacc.Bacc("TRN2", target_bir_lowering=False, debug=False)

    feats_in = nc.dram_tensor("feats", [BL, F], dt.float32, kind="ExternalInput")
    rp_in = nc.dram_tensor("rp", [F, DP], dt.bfloat16, kind="ExternalInput")
    protot_in = nc.dram_tensor("protot", [DP, CP], dt.bfloat16, kind="ExternalInput")
    # ppm[:, :C] = prototype popcounts, ppm[:, C:] = class_counts>0 mask,
    # both replicated across the 128 partitions on the host.
    ppm_in = nc.dram_tensor("ppm", [128, 2 * C], dt.float32, kind="ExternalInput")
    preds_out = nc.dram_tensor("preds", [BL, 1], dt.int32, kind="ExternalOutput")
    sims_out = nc.dram_tensor("sims", [BL, C], dt.float32, kind="ExternalOutput")

    with tile.TileContext(nc) as tc, ExitStack() as ctx:
        const = ctx.enter_context(tc.tile_pool(name="const", bufs=1))
        ident = const.tile([128, 128], dt.float32)
        make_identity(nc, ident[:])
        ppm = const.tile([128, 2 * C], dt.float32)
        nc.sync.dma_start(ppm[:], ppm_in[:])

        # random_projection as lhsT chunks [F-part, D], pieces of RPC d-chunks
        # so the first matmuls only wait on the first piece of each f-chunk.
        rp_pool = ctx.enter_context(tc.tile_pool(name="rp", bufs=1))
        rp_sb = [[None] * NRPC for _ in range(NF)]

        def emit_rp_dma(piece):
            lo_d = piece * RPC
            w = min(RPC, ND - lo_d) * 128
            for f in range(NF):
                t = rp_pool.tile(
                    [128, w], dt.bfloat16, tag=f"rp{f}_{piece}", name=f"rp{f}_{piece}"
                )
                nc.sync.dma_start(
                    t[:], rp_in[f * 128 : (f + 1) * 128, lo_d * 128 : lo_d * 128 + w]
                )
                rp_sb[f][piece] = t

        def rp_slice(f, d):
            t = rp_sb[f][d // RPC]
            r = d % RPC
            return t[:, r * 128 : (r + 1) * 128]

        # features for one bsuper: load [128, F] row blocks, PE-transpose to
        # [F, rows], split into bf16 hi/lo
        fst_pool = ctx.enter_context(tc.tile_pool(name="fst", bufs=6))
        tp_psum = ctx.enter_context(tc.tile_pool(name="tpp", bufs=2, space="PSUM"))
        ft_pool = ctx.enter_context(tc.tile_pool(name="ft", bufs=2))
        ft_hi = [[None] * NBS for _ in range(NF)]
        ft_lo = [[None] * NBS for _ in range(NF)]
        stage_tiles = [[None] * NBB for _ in range(NBS)]

        def emit_feats_dma(bs):
            for bb in range(NBB):
                gb = bs * NBB + bb
                st = fst_pool.tile([128, F], dt.float32, tag="stage", name=f"st{gb}")
                nc.sync.dma_start(st[:], feats_in[gb * 128 : (gb + 1) * 128, :])
                stage_tiles[bs][bb] = st

        def emit_feats_transpose(bs, bb):
            st = stage_tiles[bs][bb]
            for f in range(NF):
                if bb == 0:
                    ft_hi[f][bs] = ft_pool.tile(
                        [128, BS], dt.bfloat16, tag=f"hi{f}", name=f"hi{f}_{bs}"
                    )
                    ft_lo[f][bs] = ft_pool.tile(
                        [128, BS], dt.bfloat16, tag=f"lo{f}", name=f"lo{f}_{bs}"
                    )
                pt = tp_psum.tile([128, 128], dt.float32, tag="tps")
                nc.tensor.transpose(pt[:], st[:, f * 128 : (f + 1) * 128], ident[:])
                hi = ft_hi[f][bs][:, bb * 128 : (bb + 1) * 128]
                lo = ft_lo[f][bs][:, bb * 128 : (bb + 1) * 128]
                nc.vector.tensor_copy(hi, pt[:])
                nc.vector.tensor_tensor(lo, pt[:], hi, op.subtract)

        emit_feats_dma(0)
        emit_rp_dma(0)
        for bb in range(NBB):
            emit_feats_transpose(0, bb)

        # prototypes^T, all 79 chunks resident: [128, 79*101] bf16
        pt_pool = ctx.enter_context(tc.tile_pool(name="pt", bufs=1))
        protot_sb = pt_pool.tile([128, ND * CP], dt.bfloat16)
        nc.sync.dma_start(
            protot_sb[:].rearrange("p (n c) -> p n c", c=CP),
            protot_in[:].rearrange("(n p) c -> p n c", p=128),
        )
        for piece in range(1, NRPC):
            emit_rp_dma(piece)

        hv_pool = ctx.enter_context(tc.tile_pool(name="hv", bufs=3))
        pj_psum = ctx.enter_context(tc.tile_pool(name="pj", bufs=3, space="PSUM"))
        sm_psum = ctx.enter_context(tc.tile_pool(name="sm", bufs=2, space="PSUM"))
        ev_pool = ctx.enter_context(tc.tile_pool(name="ev", bufs=3))
        out_pool = ctx.enter_context(tc.tile_pool(name="out", bufs=2))

        def emit_post(bs, sim_acc):
            """similarities + argmax for a finished bsuper accumulator."""
            r0 = bs * BS
            evt = ev_pool.tile([CP, BS], dt.float32, tag="ev", name=f"ev{bs}")
            nc.scalar.copy(evt[:], sim_acc[:])
            for bb in range(NBB):
                tp = tp_psum.tile([128, CP], dt.float32, tag="tps")
                nc.tensor.transpose(
                    tp[:], evt[:, bb * 128 : (bb + 1) * 128], ident[:CP, :CP]
                )
                u = out_pool.tile([128, C], dt.float32, tag="u")
                # u = 2*cross - proto_pop   (exact small integers in fp32)
                nc.vector.scalar_tensor_tensor(
                    u[:], tp[:, 0:C], 2.0, ppm[:, 0:C], op.mult, op.subtract
                )
                s2 = out_pool.tile([128, C], dt.float32, tag="s2")
                # s2 = (u - hv_pop) * 1e-4 = -hamming/10000
                nc.vector.tensor_scalar(
                    s2[:], u[:], tp[:, C : C + 1], 1e-4, op.subtract, op.mult
                )
                s3 = out_pool.tile([128, C], dt.float32, tag="s3")
                # s3 = (s2 + 1) * mask
                nc.vector.scalar_tensor_tensor(
                    s3[:], s2[:], 1.0, ppm[:, C : 2 * C], op.add, op.mult
                )
                rr = r0 + bb * 128
                nc.sync.dma_start(sims_out[rr : rr + 128, :], s3[:])
                m8 = out_pool.tile([128, 8], dt.float32, tag="m8")
                i8 = out_pool.tile([128, 8], dt.uint32, tag="i8")
                nc.vector.max(m8[:], s3[:])
                nc.vector.max_index(i8[:], m8[:], s3[:])
                nc.sync.dma_start(
                    preds_out[rr : rr + 128, :], i8[:, 0:1].bitcast(dt.int32)
                )

        pending_post = None
        for bs in range(NBS):
            r0 = bs * BS
            sim_acc = sm_psum.tile([CP, BS], dt.float32, tag="simacc", name=f"sim{bs}")
            mm2_args = None
            for d in range(ND):
                pj = pj_psum.tile([128, BS], dt.float32, tag="proj")
                for f in range(NF):
                    lhsT = rp_slice(f, d)
                    nc.tensor.matmul(
                        pj[:], lhsT, ft_hi[f][bs][:],
                        start=(f == 0), stop=False,
                    )
                    nc.tensor.matmul(
                        pj[:], lhsT, ft_lo[f][bs][:],
                        start=False, stop=(f == NF - 1),
                    )
                # skew matmul2 one d behind so the PE never waits on the
                # threshold: hv(d-1) is ready while matmul1(d) runs
                if mm2_args is not None:
                    nc.tensor.matmul(*mm2_args[0], **mm2_args[1])
                hv = hv_pool.tile([128, BS], dt.bfloat16, tag="hv")
                nc.vector.tensor_scalar(hv[:], pj[:], 0.0, None, op.is_gt)
                mm2_args = (
                    (sim_acc[:], protot_sb[:, d * CP : (d + 1) * CP], hv[:]),
                    dict(start=(d == 0), stop=(d == ND - 1)),
                )
                # previous bsuper's postprocessing, off the PE critical path
                if d == 4 and pending_post is not None:
                    emit_post(*pending_post)
                    pending_post = None
                # stage next bsuper's features while this one computes
                if bs + 1 < NBS:
                    if d == 2:
                        emit_feats_dma(bs + 1)
                    elif d >= 8 and d < 8 + 4 * NBB and (d - 8) % 4 == 0:
                        emit_feats_transpose(bs + 1, (d - 8) // 4)
            nc.tensor.matmul(*mm2_args[0], **mm2_args[1])
            pending_post = (bs, sim_acc)
        emit_post(*pending_post)

    nc.finalize()
    return nc


def _get_module():
    if "nc" not in _CACHE:
        _CACHE["nc"] = _build_module()
    return _CACHE["nc"]


def _prep_inputs(features, random_projection, prototypes, class_counts):
    feats = np.ascontiguousarray(np.asarray(features, dtype=np.float32))
    rp = np.asarray(random_projection, dtype=np.float32)
    proto = np.asarray(prototypes)
    cc = np.asarray(class_counts, dtype=np.float32)

    bf = ml_dtypes.bfloat16
    rp_pad = np.zeros((F, DP), dtype=bf)
    rp_pad[:, :D] = rp.astype(bf)  # entries are +-1: exact in bf16

    protot = np.zeros((DP, CP), dtype=bf)
    protot[:D, :C] = proto.T.astype(np.float32).astype(bf)
    protot[:, C] = bf(1.0)  # popcount column (padded hv rows are all zero)

    pp = proto.astype(np.float32).sum(axis=1)          # [C] exact integers
    mask = (cc > 0).astype(np.float32)
    ppm = np.empty((128, 2 * C), dtype=np.float32)
    ppm[:, :C] = pp[None, :]
    ppm[:, C:] = mask[None, :]

    in_maps = []
    for i in range(NCORES):
        in_maps.append(
            {
                "feats": feats[i * BL : (i + 1) * BL],
                "rp": rp_pad,
                "protot": protot,
                "ppm": ppm,
            }
        )
    return in_maps


def _ensure_ntff_hook():
    """Register the NTFF profile hook that this image's boot path omits
    (antenv.axon_hooks is absent); mirrors trn_boot._ntff_profile_via_ctypes."""
    import sys
    import types
    import ctypes
    import contextlib

    if "antenv.axon_hooks" in sys.modules:
        return
    so_path = "/opt/axon/libaxon_pjrt.so"
    try:
        lib = ctypes.CDLL(so_path)
    except OSError:
        return
    if not hasattr(lib, "axon_start_nrt_profile"):
        return
    lib.axon_start_nrt_profile.argtypes = [
        ctypes.POINTER(ctypes.c_int64),
        ctypes.c_size_t,
    ]
    lib.axon_start_nrt_profile.restype = ctypes.c_int64
    lib.axon_stop_nrt_profile.argtypes = [ctypes.c_char_p]
    lib.axon_stop_nrt_profile.restype = ctypes.c_int64

    @contextlib.contextmanager
    def _hook(output_dir, device_ids):
        import jax

        jax.devices()
        if device_ids:
            ids = (ctypes.c_int64 * len(device_ids))(*device_ids)
            rc = lib.axon_start_nrt_profile(ids, len(device_ids))
        else:
            rc = lib.axon_start_nrt_profile(None, 0)
        if rc != 0:
            raise RuntimeError(f"axon_start_nrt_profile rc={rc}")
        try:
            yield
        finally:
            n = lib.axon_stop_nrt_profile(str(output_dir).encode())
            print(f"ntff profile: {n} file(s) written to {output_dir}")

    mod = types.ModuleType("antenv.axon_hooks")
    mod.get_axon_ntff_profile_hook = lambda: _hook
    mod.set_axon_ntff_profile_hook = lambda h: None
    sys.modules["antenv.axon_hooks"] = mod


def _run(in_maps, trace=False):
    from concourse.bass_utils import run_bass_kernel_spmd

    if trace:
        _ensure_ntff_hook()
    nc = _get_module()
    res = run_bass_kernel_spmd(nc, in_maps, core_ids=list(range(NCORES)), trace=trace)
    preds = np.concatenate([r["preds"][:, 0] for r in res.results]).astype(np.int32)
    sims = np.concatenate([r["sims"] for r in res.results]).astype(np.float32)
    return (preds, sims), res


def kernel(features, random_projection, prototypes, class_counts):
    in_maps = _prep_inputs(features, random_projection, prototypes, class_counts)
    out, _ = _run(in_maps, trace=False)
    return out


def kernel_traced(features, random_projection, prototypes, class_counts):
    """Like kernel(), but also returns BassKernelResults with NTFF profile."""
    in_maps = _prep_inputs(features, random_projection, prototypes, class_counts)
    return _run(in_maps, trace=True)


# revision 9
# speedup vs baseline: 1.0430x; 1.0430x over previous
"""HDC image classifier predict: features -> binary hypervectors -> hamming
similarity vs class prototypes -> (preds, similarities).

Strategy (8 NeuronCores, data-parallel over the batch):
  - Each core gets 2048 of the 16384 feature rows; random_projection and
    prototypes are replicated.
  - projection matmul runs on the PE array as two bf16 passes (hi + lo split
    of the features) accumulating in fp32 PSUM: random_projection is exactly
    representable in bf16 (entries are +-1), so the split recovers ~fp32
    precision at bf16 speed.
  - hv bits are thresholded on the Vector engine straight out of PSUM, and a
    second PE matmul against prototypes^T (with an appended ones column for
    the popcount) accumulates hamming cross terms over all of D.
  - similarities and the argmax (Vector engine max/max_index, first-occurrence
    semantics matching jnp.argmax) are computed on-device; outputs are
    gathered on the host.
"""

from contextlib import ExitStack

import numpy as np
import ml_dtypes

B, F, D, C = 16384, 512, 10000, 100
NCORES = 8
BL = B // NCORES            # 2048 rows per core
DP = 10112                  # D padded to 79*128
ND = DP // 128              # 79 d-chunks
NF = F // 128               # 4 f-chunks of the contraction dim
NBS = 4                     # batch super-chunks per core
BS = BL // NBS              # 512 rows per super-chunk
NBB = BS // 128             # 4 row-blocks per super-chunk
CP = C + 1                  # classes + popcount column
RPC = 10                    # d-chunks per rp DMA piece
NRPC = (ND + RPC - 1) // RPC  # 8 pieces

_CACHE = {}


def _build_module():
    import concourse.tile as tile
    import concourse.mybir as mybir
    from concourse import bacc
    from concourse.masks import make_identity

    dt = mybir.dt
    op = mybir.AluOpType

    nc = bacc.Bacc("TRN2", target_bir_lowering=False, debug=False)

    feats_in = nc.dram_tensor("feats", [BL, F], dt.float32, kind="ExternalInput")
    rp_in = nc.dram_tensor("rp", [F, DP], dt.bfloat16, kind="ExternalInput")
    protot_in = nc.dram_tensor("protot", [DP, CP], dt.bfloat16, kind="ExternalInput")
    # ppm[:, :C] = prototype popcounts, ppm[:, C:] = class_counts>0 mask,
    # both replicated across the 128 partitions on the host.
    ppm_in = nc.dram_tensor("ppm", [128, 2 * C], dt.float32, kind="ExternalInput")
    preds_out = nc.dram_tensor("preds", [BL, 1], dt.int32, kind="ExternalOutput")
    sims_out = nc.dram_tensor("sims", [BL, C], dt.float32, kind="ExternalOutput")

    with tile.TileContext(nc) as tc, ExitStack() as ctx:
        const = ctx.enter_context(tc.tile_pool(name="const", bufs=1))
        ident = const.tile([128, 128], dt.float32)
        make_identity(nc, ident[:])
        ppm = const.tile([128, 2 * C], dt.float32)
        nc.sync.dma_start(ppm[:], ppm_in[:])

        # random_projection as lhsT chunks [F-part, D], pieces of RPC d-chunks
        # so the first matmuls only wait on the first piece of each f-chunk.
        rp_pool = ctx.enter_context(tc.tile_pool(name="rp", bufs=1))
        rp_sb = [[None] * NRPC for _ in range(NF)]

        def emit_rp_dma(piece):
            lo_d = piece * RPC
            w = min(RPC, ND - lo_d) * 128
            for f in range(NF):
                t = rp_pool.tile(
                    [128, w], dt.bfloat16, tag=f"rp{f}_{piece}", name=f"rp{f}_{piece}"
                )
                nc.sync.dma_start(
                    t[:], rp_in[f * 128 : (f + 1) * 128, lo_d * 128 : lo_d * 128 + w]
                )
                rp_sb[f][piece] = t

        def rp_slice(f, d):
            t = rp_sb[f][d // RPC]
            r = d % RPC
            return t[:, r * 128 : (r + 1) * 128]

        # features for one bsuper: load [128, F] row blocks, PE-transpose to
        # [F, rows], split into bf16 hi/lo
        fst_pool = ctx.enter_context(tc.tile_pool(name="fst", bufs=6))
        tp_psum = ctx.enter_context(tc.tile_pool(name="tpp", bufs=2, space="PSUM"))
        ft_pool = ctx.enter_context(tc.tile_pool(name="ft", bufs=2))
        ft_hi = [[None] * NBS for _ in range(NF)]
        ft_lo = [[None] * NBS for _ in range(NF)]
        stage_tiles = [[None] * NBB for _ in range(NBS)]

        def emit_feats_dma(bs):
            for bb in range(NBB):
                gb = bs * NBB + bb
                st = fst_pool.tile([128, F], dt.float32, tag="stage", name=f"st{gb}")
                nc.sync.dma_start(st[:], feats_in[gb * 128 : (gb + 1) * 128, :])
                stage_tiles[bs][bb] = st

        def emit_feats_transpose(bs, bb):
            st = stage_tiles[bs][bb]
            for f in range(NF):
                if bb == 0:
                    ft_hi[f][bs] = ft_pool.tile(
                        [128, BS], dt.bfloat16, tag=f"hi{f}", name=f"hi{f}_{bs}"
                    )
                    ft_lo[f][bs] = ft_pool.tile(
                        [128, BS], dt.bfloat16, tag=f"lo{f}", name=f"lo{f}_{bs}"
                    )
                pt = tp_psum.tile([128, 128], dt.float32, tag="tps")
                nc.tensor.transpose(pt[:], st[:, f * 128 : (f + 1) * 128], ident[:])
                hi = ft_hi[f][bs][:, bb * 128 : (bb + 1) * 128]
                lo = ft_lo[f][bs][:, bb * 128 : (bb + 1) * 128]
                nc.vector.tensor_copy(hi, pt[:])
                nc.vector.tensor_tensor(lo, pt[:], hi, op.subtract)

        emit_rp_dma(0)
        emit_feats_dma(0)
        for bb in range(NBB):
            emit_feats_transpose(0, bb)

        # prototypes^T, all 79 chunks resident: [128, 79*101] bf16
        pt_pool = ctx.enter_context(tc.tile_pool(name="pt", bufs=1))
        protot_sb = pt_pool.tile([128, ND * CP], dt.bfloat16)
        nc.sync.dma_start(
            protot_sb[:].rearrange("p (n c) -> p n c", c=CP),
            protot_in[:].rearrange("(n p) c -> p n c", p=128),
        )
        for piece in range(1, NRPC):
            emit_rp_dma(piece)

        hv_pool = ctx.enter_context(tc.tile_pool(name="hv", bufs=3))
        pj_psum = ctx.enter_context(tc.tile_pool(name="pj", bufs=3, space="PSUM"))
        sm_psum = ctx.enter_context(tc.tile_pool(name="sm", bufs=2, space="PSUM"))
        ev_pool = ctx.enter_context(tc.tile_pool(name="ev", bufs=3))
        out_pool = ctx.enter_context(tc.tile_pool(name="out", bufs=4))

        s3_pool = ctx.enter_context(tc.tile_pool(name="s3p", bufs=8))

        def emit_post_compute(bs, sim_acc):
            """similarities for a finished bsuper accumulator; argmax deferred."""
            r0 = bs * BS
            evt = ev_pool.tile([CP, BS], dt.float32, tag="ev", name=f"ev{bs}")
            nc.scalar.copy(evt[:], sim_acc[:])
            s3s = []
            for bb in range(NBB):
                tp = tp_psum.tile([128, CP], dt.float32, tag="tps")
                nc.tensor.transpose(
                    tp[:], evt[:, bb * 128 : (bb + 1) * 128], ident[:CP, :CP]
                )
                u = out_pool.tile([128, C], dt.float32, tag="u")
                # u = 2*cross - proto_pop   (exact small integers in fp32)
                nc.vector.scalar_tensor_tensor(
                    u[:], tp[:, 0:C], 2.0, ppm[:, 0:C], op.mult, op.subtract
                )
                s2 = out_pool.tile([128, C], dt.float32, tag="s2")
                # s2 = (u - hv_pop) * 1e-4 = -hamming/10000
                nc.vector.tensor_scalar(
                    s2[:], u[:], tp[:, C : C + 1], 1e-4, op.subtract, op.mult
                )
                s3 = s3_pool.tile([128, C], dt.float32, tag="s3", name=f"s3_{bs}_{bb}")
                # s3 = (s2 + 1) * mask
                nc.vector.scalar_tensor_tensor(
                    s3[:], s2[:], 1.0, ppm[:, C : 2 * C], op.add, op.mult
                )
                rr = r0 + bb * 128
                nc.sync.dma_start(sims_out[rr : rr + 128, :], s3[:])
                s3s.append((rr, s3))
            return s3s

        def emit_post_argmax(s3s):
            for rr, s3 in s3s:
                m8 = out_pool.tile([128, 8], dt.float32, tag="m8")
                i8 = out_pool.tile([128, 8], dt.uint32, tag="i8")
                nc.vector.max(m8[:], s3[:])
                nc.vector.max_index(i8[:], m8[:], s3[:])
                nc.sync.dma_start(
                    preds_out[rr : rr + 128, :], i8[:, 0:1].bitcast(dt.int32)
                )

        pending_post = None
        pending_argmax = None
        for bs in range(NBS):
            r0 = bs * BS
            sim_acc = sm_psum.tile([CP, BS], dt.float32, tag="simacc", name=f"sim{bs}")
            mm2_args = None
            for d in range(ND):
                pj = pj_psum.tile([128, BS], dt.float32, tag="proj")
                for f in range(NF):
                    lhsT = rp_slice(f, d)
                    nc.tensor.matmul(
                        pj[:], lhsT, ft_hi[f][bs][:],
                        start=(f == 0), stop=False,
                    )
                    nc.tensor.matmul(
                        pj[:], lhsT, ft_lo[f][bs][:],
                        start=False, stop=(f == NF - 1),
                    )
                # skew matmul2 one d behind so the PE never waits on the
                # threshold: hv(d-1) is ready while matmul1(d) runs
                if mm2_args is not None:
                    nc.tensor.matmul(*mm2_args[0], **mm2_args[1])
                hv = hv_pool.tile([128, BS], dt.bfloat16, tag="hv")
                nc.vector.tensor_scalar(hv[:], pj[:], 0.0, None, op.is_gt)
                mm2_args = (
                    (sim_acc[:], protot_sb[:, d * CP : (d + 1) * CP], hv[:]),
                    dict(start=(d == 0), stop=(d == ND - 1)),
                )
                # previous bsuper's postprocessing, off the PE critical path
                if d == 2 and pending_post is not None:
                    pending_argmax = emit_post_compute(*pending_post)
                    pending_post = None
                if d == 28 and pending_argmax is not None:
                    emit_post_argmax(pending_argmax)
                    pending_argmax = None
                # stage next bsuper's features while this one computes
                if bs + 1 < NBS:
                    if d == 2:
                        emit_feats_dma(bs + 1)
                    elif d >= 8 and d < 8 + 4 * NBB and (d - 8) % 4 == 0:
                        emit_feats_transpose(bs + 1, (d - 8) // 4)
            nc.tensor.matmul(*mm2_args[0], **mm2_args[1])
            pending_post = (bs, sim_acc)
        emit_post_argmax(emit_post_compute(*pending_post))

    nc.finalize()
    return nc


def _get_module():
    if "nc" not in _CACHE:
        _CACHE["nc"] = _build_module()
    return _CACHE["nc"]


def _prep_inputs(features, random_projection, prototypes, class_counts):
    feats = np.ascontiguousarray(np.asarray(features, dtype=np.float32))
    rp = np.asarray(random_projection, dtype=np.float32)
    proto = np.asarray(prototypes)
    cc = np.asarray(class_counts, dtype=np.float32)

    bf = ml_dtypes.bfloat16
    rp_pad = np.zeros((F, DP), dtype=bf)
    rp_pad[:, :D] = rp.astype(bf)  # entries are +-1: exact in bf16

    protot = np.zeros((DP, CP), dtype=bf)
    protot[:D, :C] = proto.T.astype(np.float32).astype(bf)
    protot[:, C] = bf(1.0)  # popcount column (padded hv rows are all zero)

    pp = proto.astype(np.float32).sum(axis=1)          # [C] exact integers
    mask = (cc > 0).astype(np.float32)
    ppm = np.empty((128, 2 * C), dtype=np.float32)
    ppm[:, :C] = pp[None, :]
    ppm[:, C:] = mask[None, :]

    in_maps = []
    for i in range(NCORES):
        in_maps.append(
            {
                "feats": feats[i * BL : (i + 1) * BL],
                "rp": rp_pad,
                "protot": protot,
                "ppm": ppm,
            }
        )
    return in_maps


def _ensure_ntff_hook():
    """Register the NTFF profile hook that this image's boot path omits
    (antenv.axon_hooks is absent); mirrors trn_boot._ntff_profile_via_ctypes."""
    import sys
    import types
    import ctypes
    import contextlib

    if "antenv.axon_hooks" in sys.modules:
        return
    so_path = "/opt/axon/libaxon_pjrt.so"
    try:
        lib = ctypes.CDLL(so_path)
    except OSError:
        return
    if not hasattr(lib, "axon_start_nrt_profile"):
        return
    lib.axon_start_nrt_profile.argtypes = [
        ctypes.POINTER(ctypes.c_int64),
        ctypes.c_size_t,
    ]
    lib.axon_start_nrt_profile.restype = ctypes.c_int64
    lib.axon_stop_nrt_profile.argtypes = [ctypes.c_char_p]
    lib.axon_stop_nrt_profile.restype = ctypes.c_int64

    @contextlib.contextmanager
    def _hook(output_dir, device_ids):
        import jax

        jax.devices()
        if device_ids:
            ids = (ctypes.c_int64 * len(device_ids))(*device_ids)
            rc = lib.axon_start_nrt_profile(ids, len(device_ids))
        else:
            rc = lib.axon_start_nrt_profile(None, 0)
        if rc != 0:
            raise RuntimeError(f"axon_start_nrt_profile rc={rc}")
        try:
            yield
        finally:
            n = lib.axon_stop_nrt_profile(str(output_dir).encode())
            print(f"ntff profile: {n} file(s) written to {output_dir}")

    mod = types.ModuleType("antenv.axon_hooks")
    mod.get_axon_ntff_profile_hook = lambda: _hook
    mod.set_axon_ntff_profile_hook = lambda h: None
    sys.modules["antenv.axon_hooks"] = mod


def _run(in_maps, trace=False):
    from concourse.bass_utils import run_bass_kernel_spmd

    if trace:
        _ensure_ntff_hook()
    nc = _get_module()
    res = run_bass_kernel_spmd(nc, in_maps, core_ids=list(range(NCORES)), trace=trace)
    preds = np.concatenate([r["preds"][:, 0] for r in res.results]).astype(np.int32)
    sims = np.concatenate([r["sims"] for r in res.results]).astype(np.float32)
    return (preds, sims), res


def kernel(features, random_projection, prototypes, class_counts):
    in_maps = _prep_inputs(features, random_projection, prototypes, class_counts)
    out, _ = _run(in_maps, trace=False)
    return out


def kernel_traced(features, random_projection, prototypes, class_counts):
    """Like kernel(), but also returns BassKernelResults with NTFF profile."""
    in_maps = _prep_inputs(features, random_projection, prototypes, class_counts)
    return _run(in_maps, trace=True)


# revision 10
# speedup vs baseline: 1.0553x; 1.0118x over previous
"""HDC image classifier predict: features -> binary hypervectors -> hamming
similarity vs class prototypes -> (preds, similarities).

Strategy (8 NeuronCores, data-parallel over the batch):
  - Each core gets 2048 of the 16384 feature rows; random_projection and
    prototypes are replicated.
  - projection matmul runs on the PE array as two bf16 passes (hi + lo split
    of the features) accumulating in fp32 PSUM: random_projection is exactly
    representable in bf16 (entries are +-1), so the split recovers ~fp32
    precision at bf16 speed.
  - hv bits are thresholded on the Vector engine straight out of PSUM, and a
    second PE matmul against prototypes^T (with an appended ones column for
    the popcount) accumulates hamming cross terms over all of D.
  - similarities and the argmax (Vector engine max/max_index, first-occurrence
    semantics matching jnp.argmax) are computed on-device; outputs are
    gathered on the host.
"""

from contextlib import ExitStack

import numpy as np
import ml_dtypes

B, F, D, C = 16384, 512, 10000, 100
NCORES = 8
BL = B // NCORES            # 2048 rows per core
DP = 10112                  # D padded to 79*128
ND = DP // 128              # 79 d-chunks
NF = F // 128               # 4 f-chunks of the contraction dim
NBS = 4                     # batch super-chunks per core
BS = BL // NBS              # 512 rows per super-chunk
NBB = BS // 128             # 4 row-blocks per super-chunk
CP = C + 1                  # classes + popcount column
RPC = 10                    # d-chunks per rp DMA piece
NRPC = (ND + RPC - 1) // RPC  # 8 pieces

_CACHE = {}


def _build_module():
    import concourse.tile as tile
    import concourse.mybir as mybir
    from concourse import bacc
    from concourse.masks import make_identity

    dt = mybir.dt
    op = mybir.AluOpType

    nc = bacc.Bacc("TRN2", target_bir_lowering=False, debug=False)

    feats_in = nc.dram_tensor("feats", [BL, F], dt.float32, kind="ExternalInput")
    rp_in = nc.dram_tensor("rp", [F, DP], dt.bfloat16, kind="ExternalInput")
    protot_in = nc.dram_tensor("protot", [DP, CP], dt.bfloat16, kind="ExternalInput")
    # ppm[:, :C] = prototype popcounts, ppm[:, C:] = class_counts>0 mask,
    # both replicated across the 128 partitions on the host.
    ppm_in = nc.dram_tensor("ppm", [128, 2 * C], dt.float32, kind="ExternalInput")
    preds_out = nc.dram_tensor("preds", [BL, 1], dt.int32, kind="ExternalOutput")
    sims_out = nc.dram_tensor("sims", [BL, C], dt.float32, kind="ExternalOutput")

    with tile.TileContext(nc) as tc, ExitStack() as ctx:
        const = ctx.enter_context(tc.tile_pool(name="const", bufs=1))
        ident = const.tile([128, 128], dt.float32)
        make_identity(nc, ident[:])
        ppm = const.tile([128, 2 * C], dt.float32)
        nc.sync.dma_start(ppm[:], ppm_in[:])

        # random_projection as lhsT chunks [F-part, D], pieces of RPC d-chunks
        # so the first matmuls only wait on the first piece of each f-chunk.
        rp_pool = ctx.enter_context(tc.tile_pool(name="rp", bufs=1))
        rp_sb = [[None] * NRPC for _ in range(NF)]

        def emit_rp_dma(piece):
            lo_d = piece * RPC
            w = min(RPC, ND - lo_d) * 128
            for f in range(NF):
                t = rp_pool.tile(
                    [128, w], dt.bfloat16, tag=f"rp{f}_{piece}", name=f"rp{f}_{piece}"
                )
                nc.sync.dma_start(
                    t[:], rp_in[f * 128 : (f + 1) * 128, lo_d * 128 : lo_d * 128 + w]
                )
                rp_sb[f][piece] = t

        def rp_slice(f, d):
            t = rp_sb[f][d // RPC]
            r = d % RPC
            return t[:, r * 128 : (r + 1) * 128]

        # features for one bsuper: load [128, F] row blocks, PE-transpose to
        # [F, rows], split into bf16 hi/lo
        fst_pool = ctx.enter_context(tc.tile_pool(name="fst", bufs=6))
        tp_psum = ctx.enter_context(tc.tile_pool(name="tpp", bufs=3, space="PSUM"))
        ft_pool = ctx.enter_context(tc.tile_pool(name="ft", bufs=2))
        ft_hi = [[None] * NBS for _ in range(NF)]
        ft_lo = [[None] * NBS for _ in range(NF)]
        stage_tiles = [[None] * NBB for _ in range(NBS)]

        def emit_feats_dma(bs):
            for bb in range(NBB):
                gb = bs * NBB + bb
                st = fst_pool.tile([128, F], dt.float32, tag="stage", name=f"st{gb}")
                nc.sync.dma_start(st[:], feats_in[gb * 128 : (gb + 1) * 128, :])
                stage_tiles[bs][bb] = st

        def emit_feats_transpose(bs, bb):
            st = stage_tiles[bs][bb]
            for f in range(NF):
                if bb == 0:
                    ft_hi[f][bs] = ft_pool.tile(
                        [128, BS], dt.bfloat16, tag=f"hi{f}", name=f"hi{f}_{bs}"
                    )
                    ft_lo[f][bs] = ft_pool.tile(
                        [128, BS], dt.bfloat16, tag=f"lo{f}", name=f"lo{f}_{bs}"
                    )
                pt = tp_psum.tile([128, 128], dt.float32, tag="tps")
                nc.tensor.transpose(pt[:], st[:, f * 128 : (f + 1) * 128], ident[:])
                hi = ft_hi[f][bs][:, bb * 128 : (bb + 1) * 128]
                lo = ft_lo[f][bs][:, bb * 128 : (bb + 1) * 128]
                nc.vector.tensor_copy(hi, pt[:])
                nc.vector.tensor_tensor(lo, pt[:], hi, op.subtract)

        emit_feats_dma(0)
        emit_rp_dma(0)
        for bb in range(NBB):
            emit_feats_transpose(0, bb)

        # prototypes^T, all 79 chunks resident: [128, 79*101] bf16
        pt_pool = ctx.enter_context(tc.tile_pool(name="pt", bufs=1))
        protot_sb = pt_pool.tile([128, ND * CP], dt.bfloat16)
        nc.sync.dma_start(
            protot_sb[:].rearrange("p (n c) -> p n c", c=CP),
            protot_in[:].rearrange("(n p) c -> p n c", p=128),
        )
        for piece in range(1, NRPC):
            emit_rp_dma(piece)

        hv_pool = ctx.enter_context(tc.tile_pool(name="hv", bufs=3))
        pj_psum = ctx.enter_context(tc.tile_pool(name="pj", bufs=3, space="PSUM"))
        sm_psum = ctx.enter_context(tc.tile_pool(name="sm", bufs=2, space="PSUM"))
        ev_pool = ctx.enter_context(tc.tile_pool(name="ev", bufs=3))
        out_pool = ctx.enter_context(tc.tile_pool(name="out", bufs=4))

        s3_pool = ctx.enter_context(tc.tile_pool(name="s3p", bufs=8))

        def emit_post_compute(bs, sim_acc):
            """similarities for a finished bsuper accumulator; argmax deferred."""
            r0 = bs * BS
            evt = ev_pool.tile([CP, BS], dt.float32, tag="ev", name=f"ev{bs}")
            nc.scalar.copy(evt[:], sim_acc[:])
            s3s = []
            for bb in range(NBB):
                tp = tp_psum.tile([128, CP], dt.float32, tag="tps")
                nc.tensor.transpose(
                    tp[:], evt[:, bb * 128 : (bb + 1) * 128], ident[:CP, :CP]
                )
                u = out_pool.tile([128, C], dt.float32, tag="u")
                # u = 2*cross - proto_pop   (exact small integers in fp32)
                nc.vector.scalar_tensor_tensor(
                    u[:], tp[:, 0:C], 2.0, ppm[:, 0:C], op.mult, op.subtract
                )
                s2 = out_pool.tile([128, C], dt.float32, tag="s2")
                # s2 = (u - hv_pop) * 1e-4 = -hamming/10000
                nc.vector.tensor_scalar(
                    s2[:], u[:], tp[:, C : C + 1], 1e-4, op.subtract, op.mult
                )
                s3 = s3_pool.tile([128, C], dt.float32, tag="s3", name=f"s3_{bs}_{bb}")
                # s3 = (s2 + 1) * mask
                nc.vector.scalar_tensor_tensor(
                    s3[:], s2[:], 1.0, ppm[:, C : 2 * C], op.add, op.mult
                )
                rr = r0 + bb * 128
                nc.sync.dma_start(sims_out[rr : rr + 128, :], s3[:])
                s3s.append((rr, s3))
            return s3s

        def emit_post_argmax(s3s):
            for rr, s3 in s3s:
                m8 = out_pool.tile([128, 8], dt.float32, tag="m8")
                i8 = out_pool.tile([128, 8], dt.uint32, tag="i8")
                nc.vector.max(m8[:], s3[:])
                nc.vector.max_index(i8[:], m8[:], s3[:])
                nc.sync.dma_start(
                    preds_out[rr : rr + 128, :], i8[:, 0:1].bitcast(dt.int32)
                )

        pending_post = None
        pending_argmax = None
        for bs in range(NBS):
            r0 = bs * BS
            sim_acc = sm_psum.tile([CP, BS], dt.float32, tag="simacc", name=f"sim{bs}")
            mm2_args = None
            for d in range(ND):
                pj = pj_psum.tile([128, BS], dt.float32, tag="proj")
                for f in range(NF):
                    lhsT = rp_slice(f, d)
                    nc.tensor.matmul(
                        pj[:], lhsT, ft_hi[f][bs][:],
                        start=(f == 0), stop=False,
                    )
                    nc.tensor.matmul(
                        pj[:], lhsT, ft_lo[f][bs][:],
                        start=False, stop=(f == NF - 1),
                    )
                # skew matmul2 one d behind so the PE never waits on the
                # threshold: hv(d-1) is ready while matmul1(d) runs
                if mm2_args is not None:
                    nc.tensor.matmul(*mm2_args[0], **mm2_args[1])
                hv = hv_pool.tile([128, BS], dt.bfloat16, tag="hv")
                nc.vector.tensor_scalar(hv[:], pj[:], 0.0, None, op.is_gt)
                mm2_args = (
                    (sim_acc[:], protot_sb[:, d * CP : (d + 1) * CP], hv[:]),
                    dict(start=(d == 0), stop=(d == ND - 1)),
                )
                # previous bsuper's postprocessing, off the PE critical path
                if d == 2 and pending_post is not None:
                    pending_argmax = emit_post_compute(*pending_post)
                    pending_post = None
                if d == 28 and pending_argmax is not None:
                    emit_post_argmax(pending_argmax)
                    pending_argmax = None
                # stage next bsuper's features while this one computes
                if bs + 1 < NBS:
                    if d == 2:
                        emit_feats_dma(bs + 1)
                    elif d >= 8 and d < 8 + 4 * NBB and (d - 8) % 4 == 0:
                        emit_feats_transpose(bs + 1, (d - 8) // 4)
            nc.tensor.matmul(*mm2_args[0], **mm2_args[1])
            pending_post = (bs, sim_acc)
        emit_post_argmax(emit_post_compute(*pending_post))

    nc.finalize()
    return nc


def _get_module():
    if "nc" not in _CACHE:
        _CACHE["nc"] = _build_module()
    return _CACHE["nc"]


def _prep_inputs(features, random_projection, prototypes, class_counts):
    feats = np.ascontiguousarray(np.asarray(features, dtype=np.float32))
    rp = np.asarray(random_projection, dtype=np.float32)
    proto = np.asarray(prototypes)
    cc = np.asarray(class_counts, dtype=np.float32)

    bf = ml_dtypes.bfloat16
    rp_pad = np.zeros((F, DP), dtype=bf)
    rp_pad[:, :D] = rp.astype(bf)  # entries are +-1: exact in bf16

    protot = np.zeros((DP, CP), dtype=bf)
    protot[:D, :C] = proto.T.astype(np.float32).astype(bf)
    protot[:, C] = bf(1.0)  # popcount column (padded hv rows are all zero)

    pp = proto.astype(np.float32).sum(axis=1)          # [C] exact integers
    mask = (cc > 0).astype(np.float32)
    ppm = np.empty((128, 2 * C), dtype=np.float32)
    ppm[:, :C] = pp[None, :]
    ppm[:, C:] = mask[None, :]

    in_maps = []
    for i in range(NCORES):
        in_maps.append(
            {
                "feats": feats[i * BL : (i + 1) * BL],
                "rp": rp_pad,
                "protot": protot,
                "ppm": ppm,
            }
        )
    return in_maps


def _ensure_ntff_hook():
    """Register the NTFF profile hook that this image's boot path omits
    (antenv.axon_hooks is absent); mirrors trn_boot._ntff_profile_via_ctypes."""
    import sys
    import types
    import ctypes
    import contextlib

    if "antenv.axon_hooks" in sys.modules:
        return
    so_path = "/opt/axon/libaxon_pjrt.so"
    try:
        lib = ctypes.CDLL(so_path)
    except OSError:
        return
    if not hasattr(lib, "axon_start_nrt_profile"):
        return
    lib.axon_start_nrt_profile.argtypes = [
        ctypes.POINTER(ctypes.c_int64),
        ctypes.c_size_t,
    ]
    lib.axon_start_nrt_profile.restype = ctypes.c_int64
    lib.axon_stop_nrt_profile.argtypes = [ctypes.c_char_p]
    lib.axon_stop_nrt_profile.restype = ctypes.c_int64

    @contextlib.contextmanager
    def _hook(output_dir, device_ids):
        import jax

        jax.devices()
        if device_ids:
            ids = (ctypes.c_int64 * len(device_ids))(*device_ids)
            rc = lib.axon_start_nrt_profile(ids, len(device_ids))
        else:
            rc = lib.axon_start_nrt_profile(None, 0)
        if rc != 0:
            raise RuntimeError(f"axon_start_nrt_profile rc={rc}")
        try:
            yield
        finally:
            n = lib.axon_stop_nrt_profile(str(output_dir).encode())
            print(f"ntff profile: {n} file(s) written to {output_dir}")

    mod = types.ModuleType("antenv.axon_hooks")
    mod.get_axon_ntff_profile_hook = lambda: _hook
    mod.set_axon_ntff_profile_hook = lambda h: None
    sys.modules["antenv.axon_hooks"] = mod


def _run(in_maps, trace=False):
    from concourse.bass_utils import run_bass_kernel_spmd

    if trace:
        _ensure_ntff_hook()
    nc = _get_module()
    res = run_bass_kernel_spmd(nc, in_maps, core_ids=list(range(NCORES)), trace=trace)
    preds = np.concatenate([r["preds"][:, 0] for r in res.results]).astype(np.int32)
    sims = np.concatenate([r["sims"] for r in res.results]).astype(np.float32)
    return (preds, sims), res


def kernel(features, random_projection, prototypes, class_counts):
    in_maps = _prep_inputs(features, random_projection, prototypes, class_counts)
    out, _ = _run(in_maps, trace=False)
    return out


def kernel_traced(features, random_projection, prototypes, class_counts):
    """Like kernel(), but also returns BassKernelResults with NTFF profile."""
    in_maps = _prep_inputs(features, random_projection, prototypes, class_counts)
    return _run(in_maps, trace=True)
